# revision 4
# baseline (speedup 1.0000x reference)
"""Fused causal attention block (qkv proj + RoPE + attention + out proj) on 8 TRN2 cores.

Sharding: data-parallel over batch (2) x tensor-parallel over heads (16 -> 4 per core).
Each core computes y_partial[b] = attn_heads_group(x[b]) @ out_w[group_rows]; the host
sums the 4 partials per batch (the out-projection "all-reduce") and stacks batches.

Design (cost-model-guided; 238.7us baseline -> 151.1us):
  - fp16 everywhere off-PSUM: halves input DMA, runs every matmul at the full
    1 cycle/row PE rate regardless of width (the >=256-wide restriction is
    fp32r-only), and unlocks 2x/4x DVE modes for elementwise work.
  - Ragged causal diag tiles: per 128-row k-tile on the diagonal, scores/exp/PV
    cover only columns [d, 512). The triangle band is masked by a constant
    [128,128] fp16 bias matmul; band and tail are separate PSUM groups so the
    bank-level accumulation-group checker stays satisfied.
  - Per-chunk streams process BOTH heads of an out-projection pair interleaved,
    with exp over packed kt-pair tiles [128,1024] (halves ACT per-instruction
    init cost) and PV lagging ~4 exp slots so exp latency never stalls PE.
  - Softmax denominator rides a ones-column in the PV stationary; normalization
    = DVE reciprocal + PE broadcast-matmul (ones x recip) + DVE multiply into
    head-pair-stacked attnT2 [128, 2, S]; odd heads stage in SBUF and DMA-shift
    to partitions 64-127 (the final chunk instead consumes the staging tile
    directly via a third out-proj matmul against wo3).
  - Software pipelining: projection/v/out-proj units are injected between
    attention k-tile groups chunk-by-chunk so the in-order PE queue (and its
    p-state clock) never goes cold; PSUM rings: score-pairs 4 banks, O 2,
    proj/out-proj shared 2.
  - DMA queues: bulk loads + swaps + y stores on SP (HWDGE), nothing on the
    Pool SWDGE path (994ns/DMA generation cost dominated a whole earlier rev).
"""

import numpy as np

S = 2048
D = 1024
H = 16
DH = 64
P = 128
HPC = 4          # heads per core
QC = 512         # q-chunk width
NQC = S // QC
NKT = S // P     # k tiles
DIN_T = D // P   # contraction tiles for projections
MBIG = -49152.0  # fp16-representable; * 0.125 = -6144 -> exp == 0.0


def _build_nc(is_causal: bool, use_kbias: bool):
    import concourse.bass as bass
    import concourse.mybir as mybir
    import concourse.tile as tile

    f32 = mybir.dt.float32
    f32r = mybir.dt.float32r
    f16 = mybir.dt.float16
    EXP = mybir.ActivationFunctionType.Exp

    nc = bass.Bass()
    wfix_sem = nc.alloc_semaphore("wfix")
    xT = nc.dram_tensor("xT", [D, S], f16, kind="ExternalInput")
    wq = nc.dram_tensor("wq", [P, DIN_T * 256], f16, kind="ExternalInput")
    wk = nc.dram_tensor("wk", [P, DIN_T * 256], f16, kind="ExternalInput")
    wv = nc.dram_tensor("wv", [P, DIN_T * 256], f16, kind="ExternalInput")
    wo = nc.dram_tensor("wo", [P, 2 * D], f16, kind="ExternalInput")
    wo3 = nc.dram_tensor("wo3", [64, D], f16, kind="ExternalInput")
    ctab = nc.dram_tensor("ctab", [P, S], f16, kind="ExternalInput")
    ttab = nc.dram_tensor("ttab", [P, S], f16, kind="ExternalInput")
    tri = nc.dram_tensor("tri", [P, P], f16, kind="ExternalInput")
    ident = nc.dram_tensor("ident", [P, P], f16, kind="ExternalInput")
    kbias = nc.dram_tensor("kbias", [1, S], f32r, kind="ExternalInput")
    y = nc.dram_tensor("y", [S, D], f32, kind="ExternalOutput")

    with tile.TileContext(nc) as tc, nc.allow_low_precision(
        reason="fp16/f32r tolerances validated against the 2e-2 rel-err budget"
    ):
        with tc.tile_pool(name="pers", bufs=1) as pers:
            xT_sb = pers.tile([P, DIN_T, S], f16, tag="xT")
            qT_sb = pers.tile([P, 2, S], f16, tag="qT")
            kT_sb = pers.tile([P, 2, S], f16, tag="kT")
            v_sb = pers.tile([P, HPC, NKT, 65], f16, tag="v")
            attnT2 = pers.tile([P, 2, S], f16, tag="attnT2")
            wq_sb = pers.tile([P, DIN_T, 256], f16, tag="wq")
            wk_sb = pers.tile([P, DIN_T, 256], f16, tag="wk")
            wv_sb = pers.tile([P, DIN_T, 256], f16, tag="wv")
            wo_sb = pers.tile([P, 2, D], f16, tag="wo")
            wo3_sb = pers.tile([64, D], f16, tag="wo3")
            c_sb = pers.tile([P, S], f16, tag="ctab")
            t_sb = pers.tile([P, S], f16, tag="ttab")
            tri_sb = pers.tile([P, P], f16, tag="tri")
            ident_sb = pers.tile([P, P], f16, tag="ident")
            ones65 = pers.tile([65, 64], f16, tag="ones65")
            if use_kbias:
                kbias_sb = pers.tile([1, S], f32r, tag="kbias")
                ones_q = pers.tile([1, QC], f32r, tag="onesq")
                nc.sync.dma_start(out=kbias_sb, in_=kbias[:, :])
                nc.vector.memset(ones_q, 1.0)

            # pipelined input loads: chunk-0 essentials first
            xTv = xT.rearrange("(k p) s -> p k s", p=P)
            wqv = wq.rearrange("p (k c) -> p k c", k=DIN_T)
            nc.sync.dma_start(out=wq_sb[:, 0:4, :], in_=wqv[:, 0:4, :])
            nc.sync.dma_start(out=xT_sb[:, 0:4, 0:QC], in_=xTv[:, 0:4, 0:QC])
            nc.sync.dma_start(out=wq_sb[:, 4:8, :], in_=wqv[:, 4:8, :])
            nc.sync.dma_start(out=xT_sb[:, 4:8, 0:QC], in_=xTv[:, 4:8, 0:QC])
            nc.sync.dma_start(out=wk_sb, in_=wk.rearrange("p (k c) -> p k c", k=DIN_T))
            nc.sync.dma_start(out=c_sb, in_=ctab[:, :])
            nc.sync.dma_start(out=t_sb, in_=ttab[:, :])
            nc.sync.dma_start(out=wv_sb, in_=wv[:, :])
            def load_x_chunk(sc):
                for lo, hi in ((0, 4), (4, 8)):
                    nc.sync.dma_start(
                        out=xT_sb[:, lo:hi, sc * QC:(sc + 1) * QC],
                        in_=xTv[:, lo:hi, sc * QC:(sc + 1) * QC],
                    )

            load_x_chunk(1)
            nc.sync.dma_start(out=tri_sb, in_=tri[:, :])
            nc.sync.dma_start(out=ident_sb, in_=ident[:, :])
            nc.sync.dma_start(out=wo_sb, in_=wo[:, :])
            nc.sync.dma_start(out=wo3_sb, in_=wo3[:, :])
            nc.vector.memset(v_sb[:, :, :, 64:65], 1.0)
            nc.vector.memset(ones65[64:65, :], 1.0)

            final_ao = {}
            with (
                tc.tile_pool(name="ropep", bufs=3) as ropep,
                tc.tile_pool(name="epool", bufs=6) as epool,
                tc.tile_pool(name="rpool", bufs=2) as rpool,
                tc.tile_pool(name="oupool", bufs=3) as oupool,
                tc.tile_pool(name="aopool", bufs=3) as aopool,
                tc.tile_pool(name="ytpool", bufs=4) as ytpool,
                tc.tile_pool(name="pyps", bufs=2, space="PSUM") as pyps,
                tc.tile_pool(name="scps", bufs=2, space="PSUM") as scps,
                tc.tile_pool(name="pvps", bufs=2, space="PSUM") as pvps,
            ):
                def emit_qk_unit(sc, dst, w_sb, X, pool=None):
                    xs = xT_sb[:, :, sc * QC:(sc + 1) * QC]
                    cs = c_sb[:, sc * QC:(sc + 1) * QC]
                    ts = t_sb[:, sc * QC:(sc + 1) * QC]
                    pq = (pool or pyps).tile(
                        [P, QC], f32, tag="py" if pool is None else "sc2"
                    )
                    for kc in range(DIN_T):
                        nc.tensor.matmul(
                            pq,
                            w_sb[:, kc, X * P:(X + 1) * P],
                            xs[:, kc, :],
                            start=(kc == 0),
                            stop=(kc == DIN_T - 1),
                        )
                    tmp = ropep.tile([P, QC], f16, tag="tmp")
                    z = ropep.tile([P, QC], f16, tag="z")
                    zs = ropep.tile([P, QC], f16, tag="zs")
                    nc.vector.tensor_mul(tmp, pq, cs)
                    nc.vector.tensor_mul(z, pq, ts)
                    for blk in (0, 64):
                        nc.sync.dma_start(
                            out=zs[blk:blk + 32, :], in_=z[blk + 32:blk + 64, :]
                        )
                        nc.sync.dma_start(
                            out=zs[blk + 32:blk + 64, :], in_=z[blk:blk + 32, :]
                        )
                    dv = dst[:, X, sc * QC:(sc + 1) * QC]
                    nc.vector.tensor_add(dv, tmp, zs)

                def emit_v_unit(sc, j, pool=None):
                    xs = xT_sb[:, :, sc * QC:(sc + 1) * QC]
                    st = sc * 4 + j
                    pv = (pool or pyps).tile(
                        [P, QC], f32, tag="py" if pool is None else "sc2"
                    )
                    pvh = pv[:, 0:256]
                    for kc in range(DIN_T):
                        nc.tensor.matmul(
                            pvh,
                            xs[:, kc, j * P:(j + 1) * P],
                            wv_sb[:, kc, :],
                            start=(kc == 0),
                            stop=(kc == DIN_T - 1),
                        )
                    nc.vector.tensor_copy(
                        v_sb[:, :, st, 0:64],
                        pvh.rearrange("p (h c) -> p h c", h=HPC),
                    )

                def qk_units(sc):
                    return [
                        lambda sc=sc, d=dst, w=w_sb, X=X: emit_qk_unit(sc, d, w, X)
                        for dst, w_sb in ((qT_sb, wq_sb), (kT_sb, wk_sb))
                        for X in range(2)
                    ]

                def v_units(sc):
                    return [lambda sc=sc, j=j: emit_v_unit(sc, j) for j in range(4)]

                def emit_outproj_unit(st, nb, act_copy=False):
                    yp = pyps.tile([P, QC], f32, tag="py")
                    if final_ao and st >= (NQC - 1) * 4:
                        nc.tensor.matmul(
                            yp,
                            attnT2[:, 0, st * P:(st + 1) * P],
                            wo_sb[:, 0, nb * QC:(nb + 1) * QC],
                            start=True, stop=False,
                        )
                        nc.tensor.matmul(
                            yp,
                            attnT2[0:64, 1, st * P:(st + 1) * P],
                            wo_sb[0:64, 1, nb * QC:(nb + 1) * QC],
                            start=False, stop=False,
                        )
                        ao = final_ao[0]
                        nc.tensor.matmul(
                            yp,
                            ao[:, (st - (NQC - 1) * 4) * P:(st - (NQC - 1) * 4 + 1) * P],
                            wo3_sb[:, nb * QC:(nb + 1) * QC],
                            start=False, stop=True,
                        )
                    else:
                        for X in range(2):
                            nc.tensor.matmul(
                                yp,
                                attnT2[:, X, st * P:(st + 1) * P],
                                wo_sb[:, X, nb * QC:(nb + 1) * QC],
                                start=(X == 0),
                                stop=(X == 1),
                            )
                    yt = ytpool.tile([P, QC], f32, tag="yt")
                    if act_copy:
                        nc.scalar.copy(out=yt, in_=yp)
                    else:
                        nc.vector.tensor_copy(yt, yp)
                    nc.sync.dma_start(
                        out=y[st * P:(st + 1) * P, nb * QC:(nb + 1) * QC], in_=yt
                    )

                def outproj_units(qc, tail=False):
                    return [
                        lambda st=qc * 4 + j, nb=nb: emit_outproj_unit(
                            st, nb, tail and nb == 1
                        )
                        for j in range(4)
                        for nb in range(2)
                    ]

                def emit_attn_pair(qc, hp, inject):
                    """Both heads of pair hp stream their k-tiles interleaved
                    (PE runs ~4 exp-slots ahead), exp over packed kt-pair tiles
                    to halve the per-instruction ACT init cost."""
                    q0 = qc * QC
                    nkt = 4 * qc + 4 if is_causal else NKT
                    ndiag = 4 if is_causal else 0
                    X = hp
                    heads = (2 * hp, 2 * hp + 1)
                    Ot = {h: pvps.tile([65, QC], f32, tag="O", name=f"O{h}") for h in heads}

                    def emit_pv(h, kt, isdiag, dd, e2, c0, w):
                        # masked e2 entries are exactly 0, so each diag PV can
                        # cover its full ragged span; start zeroes kt0's full
                        # width, stop (sim-only, bank-level) rides the last kt
                        nc.tensor.matmul(
                            Ot[h][:, dd:QC],
                            v_sb[:, h, kt, 0:65],
                            e2[:, c0:c0 + w],
                            start=(kt == 0),
                            stop=(kt == nkt - 1),
                        )

                    pend = []
                    for ktp in range(nkt // 2):
                        kts = (2 * ktp, 2 * ktp + 1)
                        spans, col0 = [], 0
                        for kt in kts:
                            d = (kt - (nkt - ndiag)) * P if kt >= nkt - ndiag else -1
                            w = QC - d if d >= 0 else QC
                            spans.append((kt, d, col0, w))
                            col0 += w
                        for h in heads:
                            o = 64 * (h % 2)
                            sc2 = scps.tile([P, 2 * QC], f32, tag="sc2")
                            for kt, d, c0, w in spans:
                                dd = max(d, 0)
                                kh = kT_sb[o:o + 64, X, kt * P:(kt + 1) * P]
                                if d < 0:
                                    nc.tensor.matmul(
                                        sc2[:, c0:c0 + w],
                                        kh,
                                        qT_sb[o:o + 64, X, q0 + dd:q0 + QC],
                                        start=True,
                                        stop=not use_kbias,
                                    )
                                    if use_kbias:
                                        nc.tensor.matmul(
                                            sc2[:, c0:c0 + w],
                                            kbias_sb[:, kt * P:(kt + 1) * P],
                                            ones_q[:, 0:w],
                                            start=False,
                                            stop=True,
                                        )
                                else:
                                    # triangle band: scores + mask bias close
                                    # one group; the clean tail is its own
                                    nc.tensor.matmul(
                                        sc2[:, c0:c0 + P],
                                        kh,
                                        qT_sb[o:o + 64, X, q0 + dd:q0 + dd + P],
                                        start=True,
                                        stop=False,
                                    )
                                    nc.tensor.matmul(
                                        sc2[:, c0:c0 + P],
                                        ident_sb,
                                        tri_sb,
                                        start=False,
                                        stop=True,
                                    )
                                    if w > P:
                                        nc.tensor.matmul(
                                            sc2[:, c0 + P:c0 + w],
                                            kh,
                                            qT_sb[o:o + 64, X, q0 + dd + P:q0 + QC],
                                            start=True,
                                            stop=True,
                                        )
                            e2 = epool.tile([P, 2 * QC], f16, tag="e2")
                            nc.scalar.activation(
                                out=e2[:, 0:col0], in_=sc2[:, 0:col0],
                                func=EXP, scale=0.125,
                            )
                            for kt, d, c0, w in spans:
                                pend.append((h, kt, d >= 0, max(d, 0), e2, c0, w))
                            while len(pend) > 8:
                                emit_pv(*pend.pop(0))
                        inject()
                    while pend:
                        emit_pv(*pend.pop(0))
                    # normalize into head-pair-stacked attnT2; copy O out of
                    # PSUM first so the bank frees before the long r-chain;
                    # interleave the two heads' chains to hide engine latency.
                    # For the final pair nothing queues behind the O banks, so
                    # skip the staging copy and run the odd head (whose DMA
                    # shift is on the out-proj critical path) first.
                    last = qc == NQC - 1 and hp == 1
                    hseq = tuple(reversed(heads)) if last else heads
                    ou, r = {}, {}
                    for h in hseq:
                        ou[h] = oupool.tile([65, QC], f16, tag="ou", name=f"ou{h}")
                        nc.vector.tensor_copy(ou[h], Ot[h])
                    for h in hseq:
                        r[h] = rpool.tile([P, QC], f16, tag="r", name=f"r{h}")
                        nc.vector.reciprocal(r[h][64:65, :], ou[h][64:65, :])
                    bc = {}
                    for h in hseq:
                        bc[h] = pyps.tile([P, QC], f32, tag="py", name=f"bc{h}")
                        nc.tensor.matmul(
                            bc[h][0:64, :], ones65[64:65, :], r[h][64:65, :],
                            start=True, stop=True,
                        )
                    for h in hseq:
                        if h % 2 == 0:
                            nc.vector.tensor_mul(
                                attnT2[0:64, X, q0:q0 + QC], ou[h][0:64, :], bc[h][0:64, :]
                            )
                        else:
                            ao = aopool.tile([64, QC], f16, tag="ao")
                            nc.vector.tensor_mul(ao, ou[h][0:64, :], bc[h][0:64, :])
                            if last:
                                final_ao[0] = ao
                            else:
                                nc.sync.dma_start(
                                    out=attnT2[64:P, X, q0:q0 + QC], in_=ao
                                )

                # ---- orchestrate: PE filler injected into each chunk's
                # attention stream so the PE queue (and its p-state) never
                # goes cold while ACT runs exp ----
                c0_units = [
                    lambda d=dst, w=w_sb, X=X, p=pool: emit_qk_unit(0, d, w, X, p)
                    for (dst, w_sb), X, pool in [
                        ((qT_sb, wq_sb), 0, None),
                        ((qT_sb, wq_sb), 1, scps),
                        ((kT_sb, wk_sb), 0, None),
                        ((kT_sb, wk_sb), 1, scps),
                    ]
                ] + [
                    lambda j=j: emit_v_unit(0, j, scps)
                    for j in range(4)
                ]
                for u in c0_units:
                    u()
                fillers = {
                    0: qk_units(1) + v_units(1),
                    1: qk_units(2) + v_units(2) + outproj_units(0),
                    2: qk_units(3) + outproj_units(1),
                    3: v_units(3) + outproj_units(2),
                }
                for qc in range(NQC):
                    if qc + 2 < NQC:
                        load_x_chunk(qc + 2)
                    pending = fillers[qc] if is_causal else fillers.get(qc, [])
                    if not is_causal:
                        pending = (
                            (qk_units(qc + 1) + v_units(qc + 1)) if qc + 1 < NQC else []
                        ) + (outproj_units(qc - 1) if qc >= 1 else [])
                    calls_left = [4 * qc + 4 if is_causal else NKT]

                    def inject():
                        n = -(-len(pending) // max(1, calls_left[0]))
                        calls_left[0] -= 1
                        for _ in range(n):
                            if pending:
                                pending.pop(0)()

                    for hp in range(2):
                        emit_attn_pair(qc, hp, inject)
                    while pending:
                        pending.pop(0)()
                for u in outproj_units(NQC - 1, tail=True):
                    u()

    _split_matmul_waits(nc, wfix_sem)
    return nc


def _split_matmul_waits(nc, wfix_sem):
    """Walrus's engine-instruction sync-wait slots are scarce (fp32r matmul
    takes exactly one; DVE/ACT structs also cap out). Leave one wait on the
    instruction and move the rest onto NoOps inserted just before it, each
    carrying a single wait."""
    import concourse.mybir as mybir
    import bass_rust

    n_fix = 0
    for blk in nc.m.functions[0].blocks:
        il = blk.instructions
        out = []
        changed = False
        for inst in il:
            si = inst.sync_info
            if si is not None and len(si.on_wait) > 1:
                merged = {}
                for w in si.on_wait:
                    k = (w.sync_type, w.id, w.wait_mode)
                    if (
                        k in merged
                        and w.wait_mode == "sem-ge-imm"
                        and w.wait_reg is None
                    ):
                        if w.wait_value > merged[k].wait_value:
                            merged[k] = w
                    elif k in merged:
                        merged[(k, len(merged))] = w
                    else:
                        merged[k] = w
                waits = list(merged.values())
                if len(waits) == 1:
                    si.on_wait = waits
                    out.append(inst)
                    continue
                for j, w in enumerate(waits[:-1]):
                    nop = mybir.InstNoOp(name=f"{inst.name}-wfix{j}")
                    nop.engine = inst.engine
                    upd = bass_rust.SyncUpdate(
                        sync_type="semaphore", id=wfix_sem.num,
                        ant_name=wfix_sem.name, update_mode="sem-inc",
                        update_value=1, update_reg=None,
                    )
                    nop.sync_info = bass_rust.SyncInfo(on_wait=[w], on_update=[upd])
                    out.append(nop)
                    n_fix += 1
                si.on_wait = [waits[-1]]
                changed = True
            out.append(inst)
        if changed:
            blk.instructions = out


def _host_tables():
    j = np.arange(32)
    inv_freq = (10000.0 ** (-j / 32.0)).astype(np.float64)
    ang = np.arange(S, dtype=np.float64)[:, None] * inv_freq[None, :]  # [S, 32]
    cosv = np.cos(ang).astype(np.float32).T   # [32, S]
    sinv = np.sin(ang).astype(np.float32).T
    C = np.empty((P, S), dtype=np.float32)
    T = np.empty((P, S), dtype=np.float32)
    for blk in (0, 64):
        C[blk:blk + 32] = cosv
        C[blk + 32:blk + 64] = cosv
        T[blk:blk + 32] = sinv          # lo rows carry +sin (headed to hi output)
        T[blk + 32:blk + 64] = -sinv    # hi rows carry -sin (headed to lo output)
    i = np.arange(P)[:, None]
    c = np.arange(P)[None, :]
    TRI = np.where(c >= i, 0.0, MBIG).astype(np.float16)
    return C.astype(np.float16), T.astype(np.float16), TRI


def _wtile(w):
    # [D, 256] -> [128, DIN_T*256] (contraction-tile-major rows)
    return np.ascontiguousarray(
        w.reshape(DIN_T, P, 256).transpose(1, 0, 2).reshape(P, DIN_T * 256)
    )


def _in_maps(x, qkv_w, out_w, attn_mask, is_causal):
    C, T, TRI = _host_tables()
    ident = np.eye(P, dtype=np.float16)
    wq_full = qkv_w[:, 0:D]
    wk_full = qkv_w[:, D:2 * D]
    wv_full = qkv_w[:, 2 * D:3 * D]
    use_kbias = (not is_causal) and not bool(np.all(attn_mask))
    maps = []
    for core in range(8):
        b, hg = core // 4, core % 4
        cols = slice(hg * 256, (hg + 1) * 256)
        if use_kbias:
            kb = np.where(attn_mask[b], 0.0, -240000.0).astype(np.float32)[None, :]
        else:
            kb = np.zeros((1, S), dtype=np.float32)
        # wo: [128, 2, D] head-pair stacked rows: row 64r+v, pair X = head 2X+r
        wo_g = out_w[hg * 256:(hg + 1) * 256, :].reshape(2, 2, 64, D)  # (X, r, v, d)
        wo2 = np.ascontiguousarray(wo_g.transpose(1, 2, 0, 3).reshape(P, 2 * D))
        wo3 = np.ascontiguousarray(out_w[hg * 256 + 192:hg * 256 + 256, :])
        maps.append(
            dict(
                xT=np.ascontiguousarray(x[b].T).astype(np.float16),
                wq=_wtile(wq_full[:, cols]).astype(np.float16),
                wk=_wtile(wk_full[:, cols]).astype(np.float16),
                wv=_wtile(wv_full[:, cols]).astype(np.float16),
                wo=wo2.astype(np.float16),
                wo3=wo3.astype(np.float16),
                ctab=C,
                ttab=T,
                tri=TRI,
                ident=ident,
                kbias=kb,
            )
        )
    return maps, use_kbias


def kernel(x, qkv_w, out_w, attn_mask, is_causal):
    from concourse.bass_utils import run_bass_kernel_spmd

    x = np.asarray(x, dtype=np.float32)
    qkv_w = np.asarray(qkv_w, dtype=np.float32)
    out_w = np.asarray(out_w, dtype=np.float32)
    attn_mask = np.asarray(attn_mask).astype(bool)
    causal = bool(np.asarray(is_causal))

    maps, use_kbias = _in_maps(x, qkv_w, out_w, attn_mask, causal)
    nc = _build_nc(causal, use_kbias)
    res = run_bass_kernel_spmd(nc, maps, list(range(8)))
    out = np.zeros((2, S, D), dtype=np.float32)
    for core in range(8):
        out[core // 4] += res.results[core]["y"]
    return out


# revision 8
# speedup vs baseline: 1.0232x; 1.0232x over previous
"""Fused causal attention block (qkv proj + RoPE + attention + out proj) on 8 TRN2 cores.

Sharding: data-parallel over batch (2) x tensor-parallel over heads (16 -> 4 per core).
Each core computes y_partial[b] = attn_heads_group(x[b]) @ out_w[group_rows]; the host
sums the 4 partials per batch (the out-projection "all-reduce") and stacks batches.

Design (cost-model-guided; 238.7us baseline -> 151.1us):
  - fp16 everywhere off-PSUM: halves input DMA, runs every matmul at the full
    1 cycle/row PE rate regardless of width (the >=256-wide restriction is
    fp32r-only), and unlocks 2x/4x DVE modes for elementwise work.
  - Ragged causal diag tiles: per 128-row k-tile on the diagonal, scores/exp/PV
    cover only columns [d, 512). The triangle band is masked by a constant
    [128,128] fp16 bias matmul; band and tail are separate PSUM groups so the
    bank-level accumulation-group checker stays satisfied.
  - Per-chunk streams process BOTH heads of an out-projection pair interleaved,
    with exp over packed kt-pair tiles [128,1024] (halves ACT per-instruction
    init cost) and PV lagging ~4 exp slots so exp latency never stalls PE.
  - Softmax denominator rides a ones-column in the PV stationary; normalization
    = DVE reciprocal + PE broadcast-matmul (ones x recip) + DVE multiply into
    head-pair-stacked attnT2 [128, 2, S]; odd heads stage in SBUF and DMA-shift
    to partitions 64-127 (the final chunk instead consumes the staging tile
    directly via a third out-proj matmul against wo3).
  - Software pipelining: projection/v/out-proj units are injected between
    attention k-tile groups chunk-by-chunk so the in-order PE queue (and its
    p-state clock) never goes cold; PSUM rings: score-pairs 4 banks, O 2,
    proj/out-proj shared 2.
  - DMA queues: bulk loads + swaps + y stores on SP (HWDGE), nothing on the
    Pool SWDGE path (994ns/DMA generation cost dominated a whole earlier rev).
"""

import numpy as np

S = 2048
D = 1024
H = 16
DH = 64
P = 128
HPC = 4          # heads per core
QC = 512         # q-chunk width
NQC = S // QC
NKT = S // P     # k tiles
DIN_T = D // P   # contraction tiles for projections
MBIG = -49152.0  # fp16-representable; * 0.125 = -6144 -> exp == 0.0


def _build_nc(is_causal: bool, use_kbias: bool):
    import concourse.bass as bass
    import concourse.mybir as mybir
    import concourse.tile as tile

    f32 = mybir.dt.float32
    f32r = mybir.dt.float32r
    f16 = mybir.dt.float16
    EXP = mybir.ActivationFunctionType.Exp

    nc = bass.Bass()
    wfix_sem = nc.alloc_semaphore("wfix")
    xT = nc.dram_tensor("xT", [D, S], f16, kind="ExternalInput")
    wq = nc.dram_tensor("wq", [P, DIN_T * 256], f16, kind="ExternalInput")
    wk = nc.dram_tensor("wk", [P, DIN_T * 256], f16, kind="ExternalInput")
    wv = nc.dram_tensor("wv", [P, DIN_T * 256], f16, kind="ExternalInput")
    wo = nc.dram_tensor("wo", [P, 2 * D], f16, kind="ExternalInput")
    wo3 = nc.dram_tensor("wo3", [64, D], f16, kind="ExternalInput")
    ctab = nc.dram_tensor("ctab", [P, S], f16, kind="ExternalInput")
    ttab = nc.dram_tensor("ttab", [P, S], f16, kind="ExternalInput")
    tri = nc.dram_tensor("tri", [P, P], f16, kind="ExternalInput")
    ident = nc.dram_tensor("ident", [P, P], f16, kind="ExternalInput")
    kbias = nc.dram_tensor("kbias", [1, S], f32r, kind="ExternalInput")
    y = nc.dram_tensor("y", [S, D], f16, kind="ExternalOutput")

    with tile.TileContext(nc) as tc, nc.allow_low_precision(
        reason="fp16/f32r tolerances validated against the 2e-2 rel-err budget"
    ):
        with tc.tile_pool(name="pers", bufs=1) as pers:
            xT_sb = pers.tile([P, DIN_T, S], f16, tag="xT")
            qT_sb = pers.tile([P, 2, S], f16, tag="qT")
            kT_sb = pers.tile([P, 2, S], f16, tag="kT")
            v_sb = pers.tile([P, HPC, NKT, 65], f16, tag="v")
            attnT2 = pers.tile([P, 2, S], f16, tag="attnT2")
            wq_sb = pers.tile([P, DIN_T, 256], f16, tag="wq")
            wk_sb = pers.tile([P, DIN_T, 256], f16, tag="wk")
            wv_sb = pers.tile([P, DIN_T, 256], f16, tag="wv")
            wo_sb = pers.tile([P, 2, D], f16, tag="wo")
            wo3_sb = pers.tile([64, D], f16, tag="wo3")
            c_sb = pers.tile([P, S], f16, tag="ctab")
            t_sb = pers.tile([P, S], f16, tag="ttab")
            tri_sb = pers.tile([P, P], f16, tag="tri")
            ident_sb = pers.tile([P, P], f16, tag="ident")
            ones65 = pers.tile([65, 64], f16, tag="ones65")
            if use_kbias:
                kbias_sb = pers.tile([1, S], f32r, tag="kbias")
                ones_q = pers.tile([1, QC], f32r, tag="onesq")
                nc.sync.dma_start(out=kbias_sb, in_=kbias[:, :])
                nc.vector.memset(ones_q, 1.0)

            # pipelined input loads: chunk-0 essentials first
            xTv = xT.rearrange("(k p) s -> p k s", p=P)
            wqv = wq.rearrange("p (k c) -> p k c", k=DIN_T)
            nc.sync.dma_start(out=wq_sb[:, 0:4, :], in_=wqv[:, 0:4, :])
            nc.sync.dma_start(out=xT_sb[:, 0:4, 0:QC], in_=xTv[:, 0:4, 0:QC])
            nc.sync.dma_start(out=wq_sb[:, 4:8, :], in_=wqv[:, 4:8, :])
            nc.sync.dma_start(out=xT_sb[:, 4:8, 0:QC], in_=xTv[:, 4:8, 0:QC])
            nc.sync.dma_start(out=wk_sb, in_=wk.rearrange("p (k c) -> p k c", k=DIN_T))
            nc.sync.dma_start(out=c_sb, in_=ctab[:, :])
            nc.sync.dma_start(out=t_sb, in_=ttab[:, :])
            nc.sync.dma_start(out=wv_sb, in_=wv[:, :])
            def load_x_chunk(sc):
                for lo, hi in ((0, 4), (4, 8)):
                    nc.sync.dma_start(
                        out=xT_sb[:, lo:hi, sc * QC:(sc + 1) * QC],
                        in_=xTv[:, lo:hi, sc * QC:(sc + 1) * QC],
                    )

            load_x_chunk(1)
            nc.sync.dma_start(out=tri_sb, in_=tri[:, :])
            nc.sync.dma_start(out=ident_sb, in_=ident[:, :])
            nc.sync.dma_start(out=wo_sb, in_=wo[:, :])
            nc.sync.dma_start(out=wo3_sb, in_=wo3[:, :])
            nc.vector.memset(v_sb[:, :, :, 64:65], 1.0)
            nc.vector.memset(ones65[64:65, :], 1.0)

            final_ao = {}
            with (
                tc.tile_pool(name="ropep", bufs=3) as ropep,
                tc.tile_pool(name="epool", bufs=6) as epool,
                tc.tile_pool(name="rpool", bufs=3) as rpool,
                tc.tile_pool(name="oupool", bufs=3) as oupool,
                tc.tile_pool(name="aopool", bufs=3) as aopool,
                tc.tile_pool(name="ytpool", bufs=6) as ytpool,
                tc.tile_pool(name="pyps", bufs=2, space="PSUM") as pyps,
                tc.tile_pool(name="scps", bufs=2, space="PSUM") as scps,
                tc.tile_pool(name="pvps", bufs=2, space="PSUM") as pvps,
            ):
                def emit_qk_unit(sc, dst, w_sb, X, pool=None):
                    xs = xT_sb[:, :, sc * QC:(sc + 1) * QC]
                    cs = c_sb[:, sc * QC:(sc + 1) * QC]
                    ts = t_sb[:, sc * QC:(sc + 1) * QC]
                    pq = (pool or pyps).tile(
                        [P, QC], f32, tag="py" if pool is None else "sc2"
                    )
                    for kc in range(DIN_T):
                        nc.tensor.matmul(
                            pq,
                            w_sb[:, kc, X * P:(X + 1) * P],
                            xs[:, kc, :],
                            start=(kc == 0),
                            stop=(kc == DIN_T - 1),
                        )
                    tmp = ropep.tile([P, QC], f16, tag="tmp")
                    z = ropep.tile([P, QC], f16, tag="z")
                    zs = ropep.tile([P, QC], f16, tag="zs")
                    nc.vector.tensor_mul(tmp, pq, cs)
                    nc.vector.tensor_mul(z, pq, ts)
                    for blk in (0, 64):
                        nc.sync.dma_start(
                            out=zs[blk:blk + 32, :], in_=z[blk + 32:blk + 64, :]
                        )
                        nc.sync.dma_start(
                            out=zs[blk + 32:blk + 64, :], in_=z[blk:blk + 32, :]
                        )
                    dv = dst[:, X, sc * QC:(sc + 1) * QC]
                    nc.vector.tensor_add(dv, tmp, zs)

                def emit_v_unit(sc, j, pool=None):
                    xs = xT_sb[:, :, sc * QC:(sc + 1) * QC]
                    st = sc * 4 + j
                    pv = (pool or pyps).tile(
                        [P, QC], f32, tag="py" if pool is None else "sc2"
                    )
                    pvh = pv[:, 0:256]
                    for kc in range(DIN_T):
                        nc.tensor.matmul(
                            pvh,
                            xs[:, kc, j * P:(j + 1) * P],
                            wv_sb[:, kc, :],
                            start=(kc == 0),
                            stop=(kc == DIN_T - 1),
                        )
                    nc.vector.tensor_copy(
                        v_sb[:, :, st, 0:64],
                        pvh.rearrange("p (h c) -> p h c", h=HPC),
                    )

                def qk_units(sc):
                    return [
                        lambda sc=sc, d=dst, w=w_sb, X=X: emit_qk_unit(sc, d, w, X)
                        for dst, w_sb in ((qT_sb, wq_sb), (kT_sb, wk_sb))
                        for X in range(2)
                    ]

                def v_units(sc):
                    return [lambda sc=sc, j=j: emit_v_unit(sc, j) for j in range(4)]

                def emit_outproj_unit(st, nb, act_copy=False):
                    yp = pyps.tile([P, QC], f32, tag="py")
                    if final_ao and st >= (NQC - 1) * 4:
                        nc.tensor.matmul(
                            yp,
                            attnT2[:, 0, st * P:(st + 1) * P],
                            wo_sb[:, 0, nb * QC:(nb + 1) * QC],
                            start=True, stop=False,
                        )
                        nc.tensor.matmul(
                            yp,
                            attnT2[0:64, 1, st * P:(st + 1) * P],
                            wo_sb[0:64, 1, nb * QC:(nb + 1) * QC],
                            start=False, stop=False,
                        )
                        ao = final_ao[0]
                        nc.tensor.matmul(
                            yp,
                            ao[:, (st - (NQC - 1) * 4) * P:(st - (NQC - 1) * 4 + 1) * P],
                            wo3_sb[:, nb * QC:(nb + 1) * QC],
                            start=False, stop=True,
                        )
                    else:
                        for X in range(2):
                            nc.tensor.matmul(
                                yp,
                                attnT2[:, X, st * P:(st + 1) * P],
                                wo_sb[:, X, nb * QC:(nb + 1) * QC],
                                start=(X == 0),
                                stop=(X == 1),
                            )
                    yt = ytpool.tile([P, QC], f16, tag="yt")
                    if act_copy:
                        nc.scalar.copy(out=yt, in_=yp)
                    else:
                        nc.vector.tensor_copy(yt, yp)
                    nc.sync.dma_start(
                        out=y[st * P:(st + 1) * P, nb * QC:(nb + 1) * QC], in_=yt
                    )

                def outproj_units(qc, tail=False):
                    return [
                        lambda st=qc * 4 + j, nb=nb: emit_outproj_unit(
                            st, nb, tail and nb == 1
                        )
                        for j in range(4)
                        for nb in range(2)
                    ]

                def emit_attn_pair(qc, hp, inject):
                    """Both heads of pair hp stream their k-tiles interleaved
                    (PE runs ~4 exp-slots ahead), exp over packed kt-pair tiles
                    to halve the per-instruction ACT init cost."""
                    q0 = qc * QC
                    nkt = 4 * qc + 4 if is_causal else NKT
                    ndiag = 4 if is_causal else 0
                    X = hp
                    heads = (2 * hp, 2 * hp + 1)
                    Ot = {h: pvps.tile([65, QC], f32, tag="O", name=f"O{h}") for h in heads}

                    def emit_pv(h, kt, isdiag, dd, e2, c0, w):
                        # masked e2 entries are exactly 0, so each diag PV can
                        # cover its full ragged span; start zeroes kt0's full
                        # width, stop (sim-only, bank-level) rides the last kt
                        nc.tensor.matmul(
                            Ot[h][:, dd:QC],
                            v_sb[:, h, kt, 0:65],
                            e2[:, c0:c0 + w],
                            start=(kt == 0),
                            stop=(kt == nkt - 1),
                        )

                    pend = []
                    for ktp in range(nkt // 2):
                        kts = (2 * ktp, 2 * ktp + 1)
                        spans, col0 = [], 0
                        for kt in kts:
                            d = (kt - (nkt - ndiag)) * P if kt >= nkt - ndiag else -1
                            w = QC - d if d >= 0 else QC
                            spans.append((kt, d, col0, w))
                            col0 += w
                        for h in heads:
                            o = 64 * (h % 2)
                            sc2 = scps.tile([P, 2 * QC], f32, tag="sc2")
                            for kt, d, c0, w in spans:
                                dd = max(d, 0)
                                kh = kT_sb[o:o + 64, X, kt * P:(kt + 1) * P]
                                if d < 0:
                                    nc.tensor.matmul(
                                        sc2[:, c0:c0 + w],
                                        kh,
                                        qT_sb[o:o + 64, X, q0 + dd:q0 + QC],
                                        start=True,
                                        stop=not use_kbias,
                                    )
                                    if use_kbias:
                                        nc.tensor.matmul(
                                            sc2[:, c0:c0 + w],
                                            kbias_sb[:, kt * P:(kt + 1) * P],
                                            ones_q[:, 0:w],
                                            start=False,
                                            stop=True,
                                        )
                                else:
                                    # triangle band: scores + mask bias close
                                    # one group; the clean tail is its own
                                    nc.tensor.matmul(
                                        sc2[:, c0:c0 + P],
                                        kh,
                                        qT_sb[o:o + 64, X, q0 + dd:q0 + dd + P],
                                        start=True,
                                        stop=False,
                                    )
                                    nc.tensor.matmul(
                                        sc2[:, c0:c0 + P],
                                        ident_sb,
                                        tri_sb,
                                        start=False,
                                        stop=True,
                                    )
                                    if w > P:
                                        nc.tensor.matmul(
                                            sc2[:, c0 + P:c0 + w],
                                            kh,
                                            qT_sb[o:o + 64, X, q0 + dd + P:q0 + QC],
                                            start=True,
                                            stop=True,
                                        )
                            e2 = epool.tile([P, 2 * QC], f16, tag="e2")
                            nc.scalar.activation(
                                out=e2[:, 0:col0], in_=sc2[:, 0:col0],
                                func=EXP, scale=0.125,
                            )
                            for kt, d, c0, w in spans:
                                pend.append((h, kt, d >= 0, max(d, 0), e2, c0, w))
                            while len(pend) > 8:
                                emit_pv(*pend.pop(0))
                        inject()
                    while pend:
                        emit_pv(*pend.pop(0))
                    # normalize into head-pair-stacked attnT2; copy O out of
                    # PSUM first so the bank frees before the long r-chain;
                    # interleave the two heads' chains to hide engine latency.
                    # For the final pair nothing queues behind the O banks, so
                    # skip the staging copy and run the odd head (whose DMA
                    # shift is on the out-proj critical path) first.
                    last = qc == NQC - 1 and hp == 1
                    hseq = tuple(reversed(heads)) if last else heads
                    ou, r = {}, {}
                    for i, h in enumerate(hseq):
                        ou[h] = oupool.tile([65, QC], f16, tag="ou", name=f"ou{h}")
                        if last and i == 1:
                            nc.scalar.copy(out=ou[h], in_=Ot[h])
                        else:
                            nc.vector.tensor_copy(ou[h], Ot[h])
                    for h in hseq:
                        r[h] = rpool.tile([P, QC], f16, tag="r", name=f"r{h}")
                        nc.vector.reciprocal(r[h][64:65, :], ou[h][64:65, :])
                    bc = {}
                    for h in hseq:
                        bc[h] = pyps.tile([P, QC], f32, tag="py", name=f"bc{h}")
                        nc.tensor.matmul(
                            bc[h][0:64, :], ones65[64:65, :], r[h][64:65, :],
                            start=True, stop=True,
                        )
                    for h in hseq:
                        if h % 2 == 0:
                            nc.vector.tensor_mul(
                                attnT2[0:64, X, q0:q0 + QC], ou[h][0:64, :], bc[h][0:64, :]
                            )
                        else:
                            ao = aopool.tile([64, QC], f16, tag="ao")
                            nc.vector.tensor_mul(ao, ou[h][0:64, :], bc[h][0:64, :])
                            if last:
                                final_ao[0] = ao
                            else:
                                nc.sync.dma_start(
                                    out=attnT2[64:P, X, q0:q0 + QC], in_=ao
                                )

                # ---- orchestrate: PE filler injected into each chunk's
                # attention stream so the PE queue (and its p-state) never
                # goes cold while ACT runs exp ----
                c0_units = [
                    lambda d=dst, w=w_sb, X=X, p=pool: emit_qk_unit(0, d, w, X, p)
                    for (dst, w_sb), X, pool in [
                        ((qT_sb, wq_sb), 0, None),
                        ((qT_sb, wq_sb), 1, scps),
                        ((kT_sb, wk_sb), 0, None),
                        ((kT_sb, wk_sb), 1, scps),
                    ]
                ] + [
                    lambda j=j: emit_v_unit(0, j, scps)
                    for j in range(4)
                ]
                for u in c0_units:
                    u()
                fillers = {
                    0: qk_units(1) + v_units(1),
                    1: [u for p in zip(qk_units(2), v_units(2)) for u in p] + outproj_units(0),
                    2: [u for p in zip(qk_units(3), outproj_units(1)[:4]) for u in p] + outproj_units(1)[4:],
                    3: v_units(3) + outproj_units(2),
                }
                for qc in range(NQC):
                    if qc + 2 < NQC:
                        load_x_chunk(qc + 2)
                    pending = fillers[qc] if is_causal else fillers.get(qc, [])
                    if not is_causal:
                        pending = (
                            (qk_units(qc + 1) + v_units(qc + 1)) if qc + 1 < NQC else []
                        ) + (outproj_units(qc - 1) if qc >= 1 else [])
                    calls_left = [4 * qc + 4 if is_causal else NKT]

                    def inject():
                        n = -(-len(pending) // max(1, calls_left[0]))
                        calls_left[0] -= 1
                        for _ in range(n):
                            if pending:
                                pending.pop(0)()

                    for hp in range(2):
                        emit_attn_pair(qc, hp, inject)
                    while pending:
                        pending.pop(0)()
                for u in outproj_units(NQC - 1, tail=True):
                    u()

    _split_matmul_waits(nc, wfix_sem)
    return nc


def _split_matmul_waits(nc, wfix_sem):
    """Walrus's engine-instruction sync-wait slots are scarce (fp32r matmul
    takes exactly one; DVE/ACT structs also cap out). Leave one wait on the
    instruction and move the rest onto NoOps inserted just before it, each
    carrying a single wait."""
    import concourse.mybir as mybir
    import bass_rust

    n_fix = 0
    for blk in nc.m.functions[0].blocks:
        il = blk.instructions
        out = []
        changed = False
        for inst in il:
            si = inst.sync_info
            if si is not None and len(si.on_wait) > 1:
                merged = {}
                for w in si.on_wait:
                    k = (w.sync_type, w.id, w.wait_mode)
                    if (
                        k in merged
                        and w.wait_mode == "sem-ge-imm"
                        and w.wait_reg is None
                    ):
                        if w.wait_value > merged[k].wait_value:
                            merged[k] = w
                    elif k in merged:
                        merged[(k, len(merged))] = w
                    else:
                        merged[k] = w
                waits = list(merged.values())
                if len(waits) == 1:
                    si.on_wait = waits
                    out.append(inst)
                    continue
                for j, w in enumerate(waits[:-1]):
                    nop = mybir.InstNoOp(name=f"{inst.name}-wfix{j}")
                    nop.engine = inst.engine
                    upd = bass_rust.SyncUpdate(
                        sync_type="semaphore", id=wfix_sem.num,
                        ant_name=wfix_sem.name, update_mode="sem-inc",
                        update_value=1, update_reg=None,
                    )
                    nop.sync_info = bass_rust.SyncInfo(on_wait=[w], on_update=[upd])
                    out.append(nop)
                    n_fix += 1
                si.on_wait = [waits[-1]]
                changed = True
            out.append(inst)
        if changed:
            blk.instructions = out


def _host_tables():
    j = np.arange(32)
    inv_freq = (10000.0 ** (-j / 32.0)).astype(np.float64)
    ang = np.arange(S, dtype=np.float64)[:, None] * inv_freq[None, :]  # [S, 32]
    cosv = np.cos(ang).astype(np.float32).T   # [32, S]
    sinv = np.sin(ang).astype(np.float32).T
    C = np.empty((P, S), dtype=np.float32)
    T = np.empty((P, S), dtype=np.float32)
    for blk in (0, 64):
        C[blk:blk + 32] = cosv
        C[blk + 32:blk + 64] = cosv
        T[blk:blk + 32] = sinv          # lo rows carry +sin (headed to hi output)
        T[blk + 32:blk + 64] = -sinv    # hi rows carry -sin (headed to lo output)
    i = np.arange(P)[:, None]
    c = np.arange(P)[None, :]
    TRI = np.where(c >= i, 0.0, MBIG).astype(np.float16)
    return C.astype(np.float16), T.astype(np.float16), TRI


def _wtile(w):
    # [D, 256] -> [128, DIN_T*256] (contraction-tile-major rows)
    return np.ascontiguousarray(
        w.reshape(DIN_T, P, 256).transpose(1, 0, 2).reshape(P, DIN_T * 256)
    )


def _in_maps(x, qkv_w, out_w, attn_mask, is_causal):
    C, T, TRI = _host_tables()
    ident = np.eye(P, dtype=np.float16)
    wq_full = qkv_w[:, 0:D]
    wk_full = qkv_w[:, D:2 * D]
    wv_full = qkv_w[:, 2 * D:3 * D]
    use_kbias = (not is_causal) and not bool(np.all(attn_mask))
    maps = []
    for core in range(8):
        b, hg = core // 4, core % 4
        cols = slice(hg * 256, (hg + 1) * 256)
        if use_kbias:
            kb = np.where(attn_mask[b], 0.0, -240000.0).astype(np.float32)[None, :]
        else:
            kb = np.zeros((1, S), dtype=np.float32)
        # wo: [128, 2, D] head-pair stacked rows: row 64r+v, pair X = head 2X+r
        wo_g = out_w[hg * 256:(hg + 1) * 256, :].reshape(2, 2, 64, D)  # (X, r, v, d)
        wo2 = np.ascontiguousarray(wo_g.transpose(1, 2, 0, 3).reshape(P, 2 * D))
        wo3 = np.ascontiguousarray(out_w[hg * 256 + 192:hg * 256 + 256, :])
        maps.append(
            dict(
                xT=np.ascontiguousarray(x[b].T).astype(np.float16),
                wq=_wtile(wq_full[:, cols]).astype(np.float16),
                wk=_wtile(wk_full[:, cols]).astype(np.float16),
                wv=_wtile(wv_full[:, cols]).astype(np.float16),
                wo=wo2.astype(np.float16),
                wo3=wo3.astype(np.float16),
                ctab=C,
                ttab=T,
                tri=TRI,
                ident=ident,
                kbias=kb,
            )
        )
    return maps, use_kbias


def kernel(x, qkv_w, out_w, attn_mask, is_causal):
    from concourse.bass_utils import run_bass_kernel_spmd

    x = np.asarray(x, dtype=np.float32)
    qkv_w = np.asarray(qkv_w, dtype=np.float32)
    out_w = np.asarray(out_w, dtype=np.float32)
    attn_mask = np.asarray(attn_mask).astype(bool)
    causal = bool(np.asarray(is_causal))

    maps, use_kbias = _in_maps(x, qkv_w, out_w, attn_mask, causal)
    nc = _build_nc(causal, use_kbias)
    res = run_bass_kernel_spmd(nc, maps, list(range(8)))
    out = np.zeros((2, S, D), dtype=np.float32)
    for core in range(8):
        out[core // 4] += res.results[core]["y"]
    return out
